# revision 30
# baseline (speedup 1.0000x reference)
"""GCN layer (out = 0.1*h + 0.9*segment_sum(h[src], dst)) on 8 trn2 NeuronCores.

Sharding: dst-node-parallel. Core c owns 6250 dst rows (degree-balanced,
strided-dealt so every 128-row dst tile gets an even edge count). Edges are
routed to the core owning their dst. Per core, edges are grouped by dst tile
and their src features gathered from HBM with dma_gather in bf16, then
aggregated into PSUM with one-hot selection matmuls:
psum[d, f] += sum_e [dstl[e]==d] * h[src[e], f]. The residual is folded in
as an extra "self" matmul with (alpha/(1-alpha))*I and the final (1-alpha)
scale applied on PSUM evacuation (f32 out).

Perf-critical choices (HW-A/B-tested):
- 4 SWDGE queues, gather calls round-robined: descgen parallelism across
  the Q7 cores (~3x vs 1 queue; the gather path is descriptor-rate bound).
- Per-core h_pad copy is row-permuted into FIRST-USE order of that core's
  (tile-major, src-sorted) access sequence; indices remapped accordingly.
  First-time reads stream ~sequentially, killing the random-HBM penalty.
  Also pushes ~90% of slots into the first int16 chunk.
- bf16 gather payloads + matmuls (halved drain bytes, FWL-fast PE);
  dstl/iota in bf16; PSUM accumulates f32; rel err ~2e-3 (tol 2e-2).
- Batched S-generation: one tensor_tensor is_equal per (tile, chunk) over
  all its columns (broadcast dstl labels vs dense iota).
- Bulk per-iteration loads of idx/dstl/h_self (h_self host-preswizzled to
  partition-major) instead of ~100 small per-supertile DMAs.

Self-contained: hardcodes all shapes; builds + compiles the Bass kernel at
call time (layout group counts depend on the edge distribution).
"""
import numpy as np

from concourse import bacc, mybir
from concourse.tile import TileContext
from concourse.bass_utils import run_bass_kernel_spmd

N = 50000
D = 128
M = 8
RPC = 6250        # dst rows per core
TILE = 128
TPC = 49          # tiles per core (6272 rows, last 22 discarded)
NPAD = 50048      # h padded rows (>= 7*6250 + 6272)
CHUNK = 32768     # src chunk boundary (int16 index limit)
ST = 7            # tiles per supertile
NST = TPC // ST
ALPHA = 0.1
SENT = 300.0      # dstl sentinel (never equals iota 0..127)
GMAX = 1024      # max indices per dma_gather call (SWDGE descriptor carveout limit)

LAST_RESULT = None  # BassKernelResults of the most recent run (for test.py)


def _balance(src, dst):
    """Balanced node -> (core, row) assignment: deal nodes (heaviest first)
    in blocks of M to the M cores, greedily equalizing per-tile cell counts
    across cores. Returns (assign_core[n], assign_row[n],
    nodes_by_core: list of (node_ids, rows))."""
    d = np.bincount(dst, minlength=N)
    order = np.argsort(-d, kind="stable")
    assign_core = np.empty(N, dtype=np.int64)
    assign_row = np.empty(N, dtype=np.int64)
    nblocks = N // M  # 6250 blocks of 8 nodes; block b -> tile b%TPC (strided
    # deal so every tile gets an even mix of degrees), pos b//TPC
    cur = np.zeros(M, dtype=np.int64)
    for b in range(nblocks):
        t, p = b % TPC, b // TPC
        if b % TILE == 0:
            cur[:] = 0
        nodes = order[b * M:(b + 1) * M]
        cores = np.argsort(cur, kind="stable")
        assign_core[nodes] = cores
        assign_row[nodes] = t * TILE + p
        cur[cores] += d[nodes]
    nodes_by_core = []
    for c in range(M):
        ids = np.nonzero(assign_core == c)[0]
        nodes_by_core.append((ids, assign_row[ids]))
    return assign_core, assign_row, nodes_by_core


def _prep(src, dst, st_size=ST):
    """Route edges to dst-owner cores; per core, renumber src rows in
    first-use order along the device read sequence (tile-major, src-sorted
    within tile) so the per-core h_pad copy is read mostly sequentially.
    Returns per-core (idx0, idx1, dstl, perm) + shared layout."""
    assign_core, assign_row, nodes_by_core = _balance(src, dst)
    core = assign_core[dst]
    row = assign_row[dst]

    # ---- per-core edge prep: first-use renumbering + per-(tile,chunk) cells
    pc = []
    counts = np.zeros((M, TPC, 2), dtype=np.int64)
    for c in range(M):
        m = core == c
        sc = src[m]
        rw = row[m]
        tg = rw // TILE
        # canonical device order: tile-major, then src ascending
        order_c = np.lexsort((sc, tg))
        sc_o, rw_o, tg_o = sc[order_c], rw[order_c], tg[order_c]
        # first-use renumbering along that order
        uniq, first_idx, inv = np.unique(sc_o, return_index=True,
                                         return_inverse=True)
        fu_rank = np.argsort(np.argsort(first_idx, kind="stable"),
                             kind="stable")
        new_src = fu_rank[inv]                     # [Ec] new row id per edge
        perm_rows = uniq[np.argsort(first_idx, kind="stable")]  # new -> orig
        ch = (new_src >= CHUNK).astype(np.int64)
        cnt = np.bincount(tg_o * 2 + ch, minlength=TPC * 2).reshape(TPC, 2)
        counts[c] = cnt
        pc.append((sc_o, rw_o, tg_o, new_src, ch, perm_rows))

    # 16-aligned per-(tile, chunk) segment sizes (max over cores, SPMD-uniform)
    n16 = ((counts.max(axis=0) + 15) // 16) * 16          # [TPC, 2]
    nst = -(-TPC // st_size)

    # ---- call/slot/column/instance layout (core-independent) ----
    slot_base = np.zeros((TPC, 2), dtype=np.int64)   # slot base within call
    call_slots = np.zeros((nst, 2), dtype=np.int64)  # slots per (st, k) call
    call_col_base = np.zeros((nst, 2), dtype=np.int64)  # col base within st
    ncols_st = np.zeros(nst, dtype=np.int64)
    # per (t, k): range of call-relative columns [c_lo, c_hi], instance base
    c_lo = np.zeros((TPC, 2), dtype=np.int64)
    c_hi = np.zeros((TPC, 2), dtype=np.int64)
    inst_base = np.zeros((TPC, 2), dtype=np.int64)
    n_inst = 0
    for st in range(nst):
        st_cols = 0
        for k in range(2):
            call_col_base[st, k] = st_cols
            s = 0
            for t in range(st * st_size, min((st + 1) * st_size, TPC)):
                slot_base[t, k] = s
                if n16[t, k] > 0:
                    c_lo[t, k] = s // TILE
                    c_hi[t, k] = (s + n16[t, k] - 1) // TILE
                    inst_base[t, k] = n_inst
                    n_inst += c_hi[t, k] - c_lo[t, k] + 1
                else:
                    c_lo[t, k], c_hi[t, k] = 0, -1
                    inst_base[t, k] = n_inst
                s += n16[t, k]
            ns = -(-s // TILE) * TILE          # round call slots to 128
            call_slots[st, k] = ns
            st_cols += ns // TILE
        ncols_st[st] = st_cols

    idx_call_base = np.zeros((nst, 2), dtype=np.int64)
    b = [0, 0]
    for st in range(nst):
        for k in range(2):
            idx_call_base[st, k] = b[k]
            b[k] += call_slots[st, k]

    per_core = []
    for c in range(M):
        sc_o, rw_o, tg_o, new_src, ch, perm_rows = pc[c]
        # rank within each (tile, chunk) cell, in new_src-ascending order:
        # edges are already sorted by (tg, new_src-ish); sort by (tg, ch,
        # new_src) to get cell-contiguous ascending runs.
        ord2 = np.lexsort((new_src, ch, tg_o))
        sc2, rw2, tg2, ns2, ch2 = (sc_o[ord2], rw_o[ord2], tg_o[ord2],
                                   new_src[ord2], ch[ord2])
        cell2 = tg2 * 2 + ch2
        cnt = counts[c].reshape(-1)
        starts = np.zeros(TPC * 2 + 1, dtype=np.int64)
        np.cumsum(cnt, out=starts[1:])
        rank2 = np.arange(len(sc2), dtype=np.int64) - starts[cell2]

        e_slot = slot_base[tg2, ch2] + rank2           # slot within call
        e_col_rel = e_slot // TILE                     # call-relative column
        e_part = e_slot % TILE
        e_inst = inst_base[tg2, ch2] + (e_col_rel - c_lo[tg2, ch2])

        flat_idx = [np.zeros(b[k], dtype=np.int16) for k in range(2)]
        for k in range(2):
            mk = ch2 == k
            pos = idx_call_base[tg2[mk] // st_size, k] + e_slot[mk]
            flat_idx[k][pos] = (ns2[mk] - k * CHUNK).astype(np.int16)

        def wrap(flat, k):
            outs = []
            for st in range(nst):
                a = int(idx_call_base[st, k])
                n = int(call_slots[st, k])
                if n == 0:
                    continue
                blk = flat[a:a + n].reshape(n // 16, 16).T
                outs.append(np.tile(blk, (8, 1)))
            if not outs:
                return np.zeros((128, 1), np.int16)
            return np.ascontiguousarray(np.concatenate(outs, axis=1))

        idx0 = wrap(flat_idx[0], 0)
        idx1 = wrap(flat_idx[1], 1)

        dstl = np.full((TILE, max(n_inst, 1)), SENT, dtype=np.float32)
        dstl[e_part, e_inst] = (rw2 - tg2 * TILE).astype(np.float32)

        per_core.append((idx0, idx1, np.ascontiguousarray(dstl), perm_rows))

    layout = dict(nodes_by_core=nodes_by_core, st_size=st_size, nst=nst,
                  n16=n16, slot_base=slot_base, call_slots=call_slots,
                  call_col_base=call_col_base, ncols_st=ncols_st,
                  idx_call_base=idx_call_base, c_lo=c_lo, c_hi=c_hi,
                  inst_base=inst_base, n_inst=n_inst)
    return per_core, layout


def _build(layout, i0_cols, i1_cols, iters=1, mode="full", sgen="ttb", evac="dve", gb=4, pb=8, sb=4, gdt=None, gmax=GMAX, nq=4, qmode="rr", spkt=True):
    call_slots = layout["call_slots"]
    call_col_base = layout["call_col_base"]
    ncols_st = layout["ncols_st"]
    idx_call_base = layout["idx_call_base"]
    c_lo, c_hi = layout["c_lo"], layout["c_hi"]
    inst_base = layout["inst_base"]
    ncols_total = int(max(layout["n_inst"], 1))
    max_ncols_st = int(ncols_st.max())
    max_cols_tk = int(max(1, (c_hi - c_lo + 1).max()))
    st_size = int(layout.get("st_size", ST))
    nst = int(layout.get("nst", NST))

    nc = bacc.Bacc(None, target_bir_lowering=False, num_swdge_queues=nq)
    gdt = gdt or mybir.dt.bfloat16
    h_pad = nc.dram_tensor("h_pad", [NPAD, D], gdt, kind="ExternalInput")
    h_self = nc.dram_tensor("h_self", [TILE, TPC * D], mybir.dt.bfloat16,
                            kind="ExternalInput")
    iota_in = nc.dram_tensor("iota", [TILE, TILE], mybir.dt.bfloat16,
                             kind="ExternalInput")
    selfsel_in = nc.dram_tensor("selfsel", [TILE, TILE], mybir.dt.bfloat16,
                                kind="ExternalInput")
    idx0_in = nc.dram_tensor("idx0", [128, i0_cols], mybir.dt.int16,
                             kind="ExternalInput")
    idx1_in = nc.dram_tensor("idx1", [128, i1_cols], mybir.dt.int16,
                             kind="ExternalInput")
    dstl_in = nc.dram_tensor("dstl", [TILE, ncols_total], mybir.dt.bfloat16,
                             kind="ExternalInput")
    out = nc.dram_tensor("out", [TPC * TILE, D], mybir.dt.bfloat16,
                         kind="ExternalOutput")

    with TileContext(nc) as tc:
        with (
            tc.tile_pool(name="const", bufs=1) as cpool,
            tc.tile_pool(name="bulk", bufs=1) as bpool,
            tc.tile_pool(name="gbuf", bufs=gb) as gpool,
            tc.tile_pool(name="sel", bufs=sb) as spool,
            tc.tile_pool(name="io", bufs=3) as iopool,
            tc.tile_pool(name="psum", bufs=pb, space="PSUM") as ppool,
        ):
            iota_t = cpool.tile([TILE, TILE], mybir.dt.bfloat16, tag="iota")
            nc.sync.dma_start(out=iota_t[:], in_=iota_in[:, :])
            selfsel_t = cpool.tile([TILE, TILE], mybir.dt.bfloat16, tag="selfsel")
            nc.sync.dma_start(out=selfsel_t[:], in_=selfsel_in[:, :])

            if iters > 1:
                loop_cm = tc.For_i(
                    0, iters, 1,
                    hint_engines=(mybir.EngineType.Pool,
                                  mybir.EngineType.PE,
                                  mybir.EngineType.DVE,
                                  mybir.EngineType.SP,
                                  mybir.EngineType.Activation))
                loop_cm.__enter__()

            hself_t = bpool.tile([TILE, TPC * D], mybir.dt.bfloat16,
                                 tag="hself")
            nc.sync.dma_start(out=hself_t[:], in_=h_self[:, :])
            idx0_t = bpool.tile([128, i0_cols], mybir.dt.int16, tag="bidx0")
            nc.sync.dma_start(out=idx0_t[:], in_=idx0_in[:, :])
            idx1_t = bpool.tile([128, i1_cols], mybir.dt.int16, tag="bidx1")
            nc.sync.dma_start(out=idx1_t[:], in_=idx1_in[:, :])
            dstl_t = bpool.tile([TILE, ncols_total], mybir.dt.bfloat16,
                                tag="bdstl")
            nc.sync.dma_start(out=dstl_t[:], in_=dstl_in[:, :])
            idx_ts = {0: idx0_t, 1: idx1_t}
            qctr = 0
            for st in range(nst):
                t_lo, t_hi = st * st_size, min((st + 1) * st_size, TPC)
                ncst = int(ncols_st[st])
                n_inst_st = sum(
                    int(c_hi[t, k] - c_lo[t, k] + 1)
                    for t in range(t_lo, t_hi) for k in range(2)
                    if c_hi[t, k] >= c_lo[t, k])
                gbuf = gpool.tile([TILE, max_ncols_st * D], gdt,
                                  tag="gbuf")
                if mode == "stream":
                    r0 = (st * 4096) % CHUNK
                    nc.sync.dma_start(
                        out=gbuf[:, :ncst * D],
                        in_=h_pad[r0:r0 + ncst * TILE, :].rearrange(
                            "(p c) d -> p (c d)", p=TILE))
                for k in (0, 1):
                    nslots = int(call_slots[st, k])
                    if nslots == 0:
                        continue
                    idx_t = idx_ts[k]
                    ibase = int(idx_call_base[st, k]) // 16
                    a = int(call_col_base[st, k])
                    src_ap = h_pad[:CHUNK, :] if k == 0 else h_pad[CHUNK:, :]
                    # dma_gather is limited to <=1024 indices per call
                    # (SWDGE descriptor carveout).
                    for p0 in range(0, nslots, gmax):
                        ps = min(gmax, nslots - p0)
                        pcols = -(-ps // TILE)
                        ac = a + p0 // TILE
                        gview = gbuf[:, ac * D:(ac + pcols) * D].rearrange(
                            "p (c d) -> p c d", d=D)
                        if mode != "stream":
                            qn = (qctr if qmode == "rr" else qctr // 2) % nq
                            nc.gpsimd.dma_gather(
                                gview, src_ap,
                                idx_t[:, ibase + p0 // 16:
                                      ibase + (p0 + ps) // 16],
                                ps, ps, D, queue_num=qn,
                                single_packet=spkt)
                        qctr += 1

                for t in range(t_lo, t_hi):
                    if mode == "gatheronly":
                        continue
                    psum = ppool.tile([TILE, TILE], mybir.dt.float32, tag="ps")
                    first = True
                    for k in range(2):
                        if c_hi[t, k] < c_lo[t, k]:
                            continue
                        nck = int(c_hi[t, k] - c_lo[t, k] + 1)
                        ji0 = int(inst_base[t, k])
                        if sgen == "ttb":
                            Sg = spool.tile([TILE, max_cols_tk * TILE],
                                            mybir.dt.bfloat16, tag="S")
                            nc.vector.tensor_tensor(
                                out=Sg[:, :nck * TILE].rearrange(
                                    "p (c d) -> p c d", d=TILE),
                                in0=dstl_t[:, ji0:ji0 + nck].to_broadcast(
                                    [TILE, nck, TILE]),
                                in1=iota_t[:].rearrange(
                                    "p (a d) -> p a d", a=1).to_broadcast(
                                    [TILE, nck, TILE]),
                                op=mybir.AluOpType.is_equal,
                            )
                        for ci, col in enumerate(
                                range(int(c_lo[t, k]), int(c_hi[t, k]) + 1)):
                            jr = int(call_col_base[st, k]) + col
                            ji = ji0 + ci
                            if sgen == "ttb":
                                S_ap = Sg[:, ci * TILE:(ci + 1) * TILE]
                            else:
                                S = spool.tile([TILE, TILE], mybir.dt.bfloat16,
                                               tag="S")
                                nc.vector.tensor_tensor(
                                    out=S[:],
                                    in0=dstl_t[:, ji:ji + 1].to_broadcast(
                                        [TILE, TILE]),
                                    in1=iota_t[:],
                                    op=mybir.AluOpType.is_equal,
                                )
                                S_ap = S[:]
                            nc.tensor.matmul(
                                out=psum[:],
                                lhsT=S_ap,
                                rhs=gbuf[:, jr * D:(jr + 1) * D],
                                start=first,
                                stop=False,
                            )
                            first = False
                    nc.tensor.matmul(
                        out=psum[:], lhsT=selfsel_t[:],
                        rhs=hself_t[:, t * D:(t + 1) * D],
                        start=first, stop=True)
                    osb = iopool.tile([TILE, D], mybir.dt.bfloat16, tag="osb")
                    if evac == "act":
                        nc.scalar.activation(
                            out=osb[:], in_=psum[:],
                            func=mybir.ActivationFunctionType.Copy,
                            scale=1.0 - ALPHA)
                    else:
                        nc.vector.tensor_scalar_mul(osb[:], psum[:], 1.0 - ALPHA)
                    nc.sync.dma_start(
                        out=out[t * TILE:(t + 1) * TILE, :], in_=osb[:])
            if iters > 1:
                loop_cm.__exit__(None, None, None)
    nc.compile()
    return nc


def build_and_inputs(h, src, dst, st_size=ST, **bkw):
    """Returns (nc, in_maps) for the 8-core SPMD kernel."""
    h = np.ascontiguousarray(np.asarray(h, dtype=np.float32))
    src = np.asarray(src).astype(np.int64)
    dst = np.asarray(dst).astype(np.int64)

    import ml_dtypes
    bf16 = ml_dtypes.bfloat16

    per_core, layout = _prep(src, dst, st_size=st_size)
    h_bf = h.astype(bf16)
    iota = np.broadcast_to(np.arange(TILE, dtype=bf16), (TILE, TILE))
    iota = np.ascontiguousarray(iota)
    selfsel = np.ascontiguousarray(
        (np.eye(TILE, dtype=np.float32) * np.float32(ALPHA / (1.0 - ALPHA))
         ).astype(bf16))

    i0_cols = max(pc[0].shape[1] for pc in per_core)
    i1_cols = max(pc[1].shape[1] for pc in per_core)
    nc = _build(layout, i0_cols, i1_cols, **bkw)

    in_maps = []
    for c in range(M):
        idx0, idx1, dstl, perm_rows = per_core[c]
        h_pad = np.zeros((NPAD, D), bf16)
        h_pad[:len(perm_rows)] = h_bf[perm_rows]
        ids, rows = layout["nodes_by_core"][c]
        h_self = np.zeros((TPC * TILE, D), bf16)
        h_self[rows] = h_bf[ids]
        h_self = np.ascontiguousarray(
            h_self.reshape(TPC, TILE, D).transpose(1, 0, 2).reshape(
                TILE, TPC * D))
        in_maps.append({
            "h_pad": h_pad,
            "h_self": h_self,
            "iota": iota,
            "selfsel": selfsel,
            "idx0": idx0,
            "idx1": idx1,
            "dstl": dstl.astype(bf16),
        })
    return nc, in_maps, layout


def kernel(h, src, dst, **_):
    global LAST_RESULT
    import os
    # NTFF tracing needs an axon hook that is absent in this environment;
    # make sure a stray BASS_TRACE can't break execution.
    os.environ["BASS_NEVER_TRACE"] = "1"
    nc, in_maps, layout = build_and_inputs(h, src, dst)
    res = run_bass_kernel_spmd(nc, in_maps, core_ids=list(range(M)))
    LAST_RESULT = res
    out = np.empty((N, D), np.float32)
    for c in range(M):
        ids, rows = layout["nodes_by_core"][c]
        out[ids] = res.results[c]["out"][rows].astype(np.float32)
    return out



# revision 31
# speedup vs baseline: 1.0946x; 1.0946x over previous
"""GCN layer (out = 0.1*h + 0.9*segment_sum(h[src], dst)) on 8 trn2 NeuronCores.

Sharding: dst-node-parallel. Core c owns 6250 dst rows (degree-balanced,
strided-dealt so every 128-row dst tile gets an even edge count). Edges are
routed to the core owning their dst. Per core, edges are grouped by dst tile
and their src features gathered from HBM with dma_gather in bf16, then
aggregated into PSUM with one-hot selection matmuls:
psum[d, f] += sum_e [dstl[e]==d] * h[src[e], f]. The residual is folded in
as an extra "self" matmul with (alpha/(1-alpha))*I and the final (1-alpha)
scale applied on PSUM evacuation (f32 out).

Perf-critical choices (HW-A/B-tested):
- 4 SWDGE queues, gather calls round-robined: descgen parallelism across
  the Q7 cores (~3x vs 1 queue; the gather path is descriptor-rate bound).
- Per-core h_pad copy is row-permuted into FIRST-USE order of that core's
  (tile-major, src-sorted) access sequence; indices remapped accordingly.
  First-time reads stream ~sequentially, killing the random-HBM penalty.
  Also pushes ~90% of slots into the first int16 chunk.
- bf16 gather payloads + matmuls (halved drain bytes, FWL-fast PE);
  dstl/iota in bf16; PSUM accumulates f32; rel err ~2e-3 (tol 2e-2).
- Batched S-generation: one tensor_tensor is_equal per (tile, chunk) over
  all its columns (broadcast dstl labels vs dense iota).
- Bulk per-iteration loads of idx/dstl/h_self (h_self host-preswizzled to
  partition-major) instead of ~100 small per-supertile DMAs.

Self-contained: hardcodes all shapes; builds + compiles the Bass kernel at
call time (layout group counts depend on the edge distribution).
"""
import numpy as np

from concourse import bacc, mybir
from concourse.tile import TileContext
from concourse.bass_utils import run_bass_kernel_spmd

N = 50000
D = 128
M = 8
RPC = 6250        # dst rows per core
TILE = 128
TPC = 49          # tiles per core (6272 rows, last 22 discarded)
NPAD = 50048      # h padded rows (>= 7*6250 + 6272)
CHUNK = 32768     # src chunk boundary (int16 index limit)
ST = 7            # tiles per supertile
NST = TPC // ST
ALPHA = 0.1
SENT = 300.0      # dstl sentinel (never equals iota 0..127)
GMAX = 1024      # max indices per dma_gather call (SWDGE descriptor carveout limit)

LAST_RESULT = None  # BassKernelResults of the most recent run (for test.py)


def _balance(src, dst):
    """Balanced node -> (core, row) assignment: deal nodes (heaviest first)
    in blocks of M to the M cores, greedily equalizing per-tile cell counts
    across cores. Returns (assign_core[n], assign_row[n],
    nodes_by_core: list of (node_ids, rows))."""
    d = np.bincount(dst, minlength=N)
    order = np.argsort(-d, kind="stable")
    assign_core = np.empty(N, dtype=np.int64)
    assign_row = np.empty(N, dtype=np.int64)
    nblocks = N // M  # 6250 blocks of 8 nodes; block b -> tile b%TPC (strided
    # deal so every tile gets an even mix of degrees), pos b//TPC
    cur = np.zeros(M, dtype=np.int64)
    for b in range(nblocks):
        t, p = b % TPC, b // TPC
        if b % TILE == 0:
            cur[:] = 0
        nodes = order[b * M:(b + 1) * M]
        cores = np.argsort(cur, kind="stable")
        assign_core[nodes] = cores
        assign_row[nodes] = t * TILE + p
        cur[cores] += d[nodes]
    nodes_by_core = []
    for c in range(M):
        ids = np.nonzero(assign_core == c)[0]
        nodes_by_core.append((ids, assign_row[ids]))
    return assign_core, assign_row, nodes_by_core


def _prep(src, dst, st_size=ST):
    """Route edges to dst-owner cores; per core, renumber src rows in
    first-use order along the device read sequence (tile-major, src-sorted
    within tile) so the per-core h_pad copy is read mostly sequentially.
    Returns per-core (idx0, idx1, dstl, perm) + shared layout."""
    assign_core, assign_row, nodes_by_core = _balance(src, dst)
    core = assign_core[dst]
    row = assign_row[dst]

    # ---- per-core edge prep: first-use renumbering + per-(tile,chunk) cells
    pc = []
    counts = np.zeros((M, TPC, 2), dtype=np.int64)
    for c in range(M):
        m = core == c
        sc = src[m]
        rw = row[m]
        tg = rw // TILE
        # canonical device order: tile-major, then src ascending
        order_c = np.lexsort((sc, tg))
        sc_o, rw_o, tg_o = sc[order_c], rw[order_c], tg[order_c]
        # first-use renumbering along that order
        uniq, first_idx, inv = np.unique(sc_o, return_index=True,
                                         return_inverse=True)
        fu_rank = np.argsort(np.argsort(first_idx, kind="stable"),
                             kind="stable")
        new_src = fu_rank[inv]                     # [Ec] new row id per edge
        perm_rows = uniq[np.argsort(first_idx, kind="stable")]  # new -> orig
        ch = (new_src >= CHUNK).astype(np.int64)
        cnt = np.bincount(tg_o * 2 + ch, minlength=TPC * 2).reshape(TPC, 2)
        counts[c] = cnt
        pc.append((sc_o, rw_o, tg_o, new_src, ch, perm_rows))

    # 16-aligned per-(tile, chunk) segment sizes (max over cores, SPMD-uniform)
    n16 = ((counts.max(axis=0) + 15) // 16) * 16          # [TPC, 2]
    nst = -(-TPC // st_size)

    # ---- call/slot/column/instance layout (core-independent) ----
    slot_base = np.zeros((TPC, 2), dtype=np.int64)   # slot base within call
    call_slots = np.zeros((nst, 2), dtype=np.int64)  # slots per (st, k) call
    call_col_base = np.zeros((nst, 2), dtype=np.int64)  # col base within st
    ncols_st = np.zeros(nst, dtype=np.int64)
    # per (t, k): range of call-relative columns [c_lo, c_hi], instance base
    c_lo = np.zeros((TPC, 2), dtype=np.int64)
    c_hi = np.zeros((TPC, 2), dtype=np.int64)
    inst_base = np.zeros((TPC, 2), dtype=np.int64)
    n_inst = 0
    for st in range(nst):
        st_cols = 0
        for k in range(2):
            call_col_base[st, k] = st_cols
            s = 0
            for t in range(st * st_size, min((st + 1) * st_size, TPC)):
                slot_base[t, k] = s
                if n16[t, k] > 0:
                    c_lo[t, k] = s // TILE
                    c_hi[t, k] = (s + n16[t, k] - 1) // TILE
                    inst_base[t, k] = n_inst
                    n_inst += c_hi[t, k] - c_lo[t, k] + 1
                else:
                    c_lo[t, k], c_hi[t, k] = 0, -1
                    inst_base[t, k] = n_inst
                s += n16[t, k]
            ns = -(-s // TILE) * TILE          # round call slots to 128
            call_slots[st, k] = ns
            st_cols += ns // TILE
        ncols_st[st] = st_cols

    idx_call_base = np.zeros((nst, 2), dtype=np.int64)
    b = [0, 0]
    for st in range(nst):
        for k in range(2):
            idx_call_base[st, k] = b[k]
            b[k] += call_slots[st, k]

    per_core = []
    for c in range(M):
        sc_o, rw_o, tg_o, new_src, ch, perm_rows = pc[c]
        # rank within each (tile, chunk) cell, in new_src-ascending order:
        # edges are already sorted by (tg, new_src-ish); sort by (tg, ch,
        # new_src) to get cell-contiguous ascending runs.
        ord2 = np.lexsort((new_src, ch, tg_o))
        sc2, rw2, tg2, ns2, ch2 = (sc_o[ord2], rw_o[ord2], tg_o[ord2],
                                   new_src[ord2], ch[ord2])
        cell2 = tg2 * 2 + ch2
        cnt = counts[c].reshape(-1)
        starts = np.zeros(TPC * 2 + 1, dtype=np.int64)
        np.cumsum(cnt, out=starts[1:])
        rank2 = np.arange(len(sc2), dtype=np.int64) - starts[cell2]

        e_slot = slot_base[tg2, ch2] + rank2           # slot within call
        e_col_rel = e_slot // TILE                     # call-relative column
        e_part = e_slot % TILE
        e_inst = inst_base[tg2, ch2] + (e_col_rel - c_lo[tg2, ch2])

        flat_idx = [np.zeros(b[k], dtype=np.int16) for k in range(2)]
        for k in range(2):
            mk = ch2 == k
            pos = idx_call_base[tg2[mk] // st_size, k] + e_slot[mk]
            flat_idx[k][pos] = (ns2[mk] - k * CHUNK).astype(np.int16)

        def wrap(flat, k):
            outs = []
            for st in range(nst):
                a = int(idx_call_base[st, k])
                n = int(call_slots[st, k])
                if n == 0:
                    continue
                blk = flat[a:a + n].reshape(n // 16, 16).T
                outs.append(np.tile(blk, (8, 1)))
            if not outs:
                return np.zeros((128, 1), np.int16)
            return np.ascontiguousarray(np.concatenate(outs, axis=1))

        idx0 = wrap(flat_idx[0], 0)
        idx1 = wrap(flat_idx[1], 1)

        dstl = np.full((TILE, max(n_inst, 1)), SENT, dtype=np.float32)
        dstl[e_part, e_inst] = (rw2 - tg2 * TILE).astype(np.float32)

        per_core.append((idx0, idx1, np.ascontiguousarray(dstl), perm_rows))

    layout = dict(nodes_by_core=nodes_by_core, st_size=st_size, nst=nst,
                  n16=n16, slot_base=slot_base, call_slots=call_slots,
                  call_col_base=call_col_base, ncols_st=ncols_st,
                  idx_call_base=idx_call_base, c_lo=c_lo, c_hi=c_hi,
                  inst_base=inst_base, n_inst=n_inst)
    return per_core, layout


def _build(layout, i0_cols, i1_cols, iters=1, mode="full", sgen="ttb", evac="dve", gb=4, pb=8, sb=4, gdt=None, gmax=GMAX, nq=4, qmode="rr", spkt=True, scratch=16384):
    call_slots = layout["call_slots"]
    call_col_base = layout["call_col_base"]
    ncols_st = layout["ncols_st"]
    idx_call_base = layout["idx_call_base"]
    c_lo, c_hi = layout["c_lo"], layout["c_hi"]
    inst_base = layout["inst_base"]
    ncols_total = int(max(layout["n_inst"], 1))
    max_ncols_st = int(ncols_st.max())
    max_cols_tk = int(max(1, (c_hi - c_lo + 1).max()))
    st_size = int(layout.get("st_size", ST))
    nst = int(layout.get("nst", NST))

    nc = bacc.Bacc(None, target_bir_lowering=False, num_swdge_queues=nq,
                   dynamic_dma_scratch_size=scratch)
    gdt = gdt or mybir.dt.bfloat16
    h_pad = nc.dram_tensor("h_pad", [NPAD, D], gdt, kind="ExternalInput")
    h_self = nc.dram_tensor("h_self", [TILE, TPC * D], mybir.dt.bfloat16,
                            kind="ExternalInput")
    iota_in = nc.dram_tensor("iota", [TILE, TILE], mybir.dt.bfloat16,
                             kind="ExternalInput")
    selfsel_in = nc.dram_tensor("selfsel", [TILE, TILE], mybir.dt.bfloat16,
                                kind="ExternalInput")
    idx0_in = nc.dram_tensor("idx0", [128, i0_cols], mybir.dt.int16,
                             kind="ExternalInput")
    idx1_in = nc.dram_tensor("idx1", [128, i1_cols], mybir.dt.int16,
                             kind="ExternalInput")
    dstl_in = nc.dram_tensor("dstl", [TILE, ncols_total], mybir.dt.bfloat16,
                             kind="ExternalInput")
    out = nc.dram_tensor("out", [TPC * TILE, D], mybir.dt.bfloat16,
                         kind="ExternalOutput")

    with TileContext(nc) as tc:
        with (
            tc.tile_pool(name="const", bufs=1) as cpool,
            tc.tile_pool(name="bulk", bufs=1) as bpool,
            tc.tile_pool(name="gbuf", bufs=gb) as gpool,
            tc.tile_pool(name="sel", bufs=sb) as spool,
            tc.tile_pool(name="io", bufs=3) as iopool,
            tc.tile_pool(name="psum", bufs=pb, space="PSUM") as ppool,
        ):
            iota_t = cpool.tile([TILE, TILE], mybir.dt.bfloat16, tag="iota")
            nc.sync.dma_start(out=iota_t[:], in_=iota_in[:, :])
            selfsel_t = cpool.tile([TILE, TILE], mybir.dt.bfloat16, tag="selfsel")
            nc.sync.dma_start(out=selfsel_t[:], in_=selfsel_in[:, :])

            if iters > 1:
                loop_cm = tc.For_i(
                    0, iters, 1,
                    hint_engines=(mybir.EngineType.Pool,
                                  mybir.EngineType.PE,
                                  mybir.EngineType.DVE,
                                  mybir.EngineType.SP,
                                  mybir.EngineType.Activation))
                loop_cm.__enter__()

            hself_t = bpool.tile([TILE, TPC * D], mybir.dt.bfloat16,
                                 tag="hself")
            nc.sync.dma_start(out=hself_t[:], in_=h_self[:, :])
            idx0_t = bpool.tile([128, i0_cols], mybir.dt.int16, tag="bidx0")
            nc.sync.dma_start(out=idx0_t[:], in_=idx0_in[:, :])
            idx1_t = bpool.tile([128, i1_cols], mybir.dt.int16, tag="bidx1")
            nc.sync.dma_start(out=idx1_t[:], in_=idx1_in[:, :])
            dstl_t = bpool.tile([TILE, ncols_total], mybir.dt.bfloat16,
                                tag="bdstl")
            nc.sync.dma_start(out=dstl_t[:], in_=dstl_in[:, :])
            idx_ts = {0: idx0_t, 1: idx1_t}
            qctr = 0
            for st in range(nst):
                t_lo, t_hi = st * st_size, min((st + 1) * st_size, TPC)
                ncst = int(ncols_st[st])
                n_inst_st = sum(
                    int(c_hi[t, k] - c_lo[t, k] + 1)
                    for t in range(t_lo, t_hi) for k in range(2)
                    if c_hi[t, k] >= c_lo[t, k])
                gbuf = gpool.tile([TILE, max_ncols_st * D], gdt,
                                  tag="gbuf")
                if mode == "stream":
                    r0 = (st * 4096) % CHUNK
                    nc.sync.dma_start(
                        out=gbuf[:, :ncst * D],
                        in_=h_pad[r0:r0 + ncst * TILE, :].rearrange(
                            "(p c) d -> p (c d)", p=TILE))
                for k in (0, 1):
                    nslots = int(call_slots[st, k])
                    if nslots == 0:
                        continue
                    idx_t = idx_ts[k]
                    ibase = int(idx_call_base[st, k]) // 16
                    a = int(call_col_base[st, k])
                    src_ap = h_pad[:CHUNK, :] if k == 0 else h_pad[CHUNK:, :]
                    # dma_gather is limited to <=1024 indices per call
                    # (SWDGE descriptor carveout).
                    for p0 in range(0, nslots, gmax):
                        ps = min(gmax, nslots - p0)
                        pcols = -(-ps // TILE)
                        ac = a + p0 // TILE
                        gview = gbuf[:, ac * D:(ac + pcols) * D].rearrange(
                            "p (c d) -> p c d", d=D)
                        if mode != "stream":
                            qn = (qctr if qmode == "rr" else qctr // 2) % nq
                            nc.gpsimd.dma_gather(
                                gview, src_ap,
                                idx_t[:, ibase + p0 // 16:
                                      ibase + (p0 + ps) // 16],
                                ps, ps, D, queue_num=qn,
                                single_packet=spkt)
                        qctr += 1

                for t in range(t_lo, t_hi):
                    if mode == "gatheronly":
                        continue
                    psum = ppool.tile([TILE, TILE], mybir.dt.float32, tag="ps")
                    first = True
                    for k in range(2):
                        if c_hi[t, k] < c_lo[t, k]:
                            continue
                        nck = int(c_hi[t, k] - c_lo[t, k] + 1)
                        ji0 = int(inst_base[t, k])
                        if sgen == "ttb":
                            Sg = spool.tile([TILE, max_cols_tk * TILE],
                                            mybir.dt.bfloat16, tag="S")
                            nc.vector.tensor_tensor(
                                out=Sg[:, :nck * TILE].rearrange(
                                    "p (c d) -> p c d", d=TILE),
                                in0=dstl_t[:, ji0:ji0 + nck].to_broadcast(
                                    [TILE, nck, TILE]),
                                in1=iota_t[:].rearrange(
                                    "p (a d) -> p a d", a=1).to_broadcast(
                                    [TILE, nck, TILE]),
                                op=mybir.AluOpType.is_equal,
                            )
                        for ci, col in enumerate(
                                range(int(c_lo[t, k]), int(c_hi[t, k]) + 1)):
                            jr = int(call_col_base[st, k]) + col
                            ji = ji0 + ci
                            if sgen == "ttb":
                                S_ap = Sg[:, ci * TILE:(ci + 1) * TILE]
                            else:
                                S = spool.tile([TILE, TILE], mybir.dt.bfloat16,
                                               tag="S")
                                nc.vector.tensor_tensor(
                                    out=S[:],
                                    in0=dstl_t[:, ji:ji + 1].to_broadcast(
                                        [TILE, TILE]),
                                    in1=iota_t[:],
                                    op=mybir.AluOpType.is_equal,
                                )
                                S_ap = S[:]
                            nc.tensor.matmul(
                                out=psum[:],
                                lhsT=S_ap,
                                rhs=gbuf[:, jr * D:(jr + 1) * D],
                                start=first,
                                stop=False,
                            )
                            first = False
                    nc.tensor.matmul(
                        out=psum[:], lhsT=selfsel_t[:],
                        rhs=hself_t[:, t * D:(t + 1) * D],
                        start=first, stop=True)
                    osb = iopool.tile([TILE, D], mybir.dt.bfloat16, tag="osb")
                    if evac == "act":
                        nc.scalar.activation(
                            out=osb[:], in_=psum[:],
                            func=mybir.ActivationFunctionType.Copy,
                            scale=1.0 - ALPHA)
                    else:
                        nc.vector.tensor_scalar_mul(osb[:], psum[:], 1.0 - ALPHA)
                    nc.sync.dma_start(
                        out=out[t * TILE:(t + 1) * TILE, :], in_=osb[:])
            if iters > 1:
                loop_cm.__exit__(None, None, None)
    nc.compile()
    return nc


def build_and_inputs(h, src, dst, st_size=ST, **bkw):
    """Returns (nc, in_maps) for the 8-core SPMD kernel."""
    h = np.ascontiguousarray(np.asarray(h, dtype=np.float32))
    src = np.asarray(src).astype(np.int64)
    dst = np.asarray(dst).astype(np.int64)

    import ml_dtypes
    bf16 = ml_dtypes.bfloat16

    per_core, layout = _prep(src, dst, st_size=st_size)
    h_bf = h.astype(bf16)
    iota = np.broadcast_to(np.arange(TILE, dtype=bf16), (TILE, TILE))
    iota = np.ascontiguousarray(iota)
    selfsel = np.ascontiguousarray(
        (np.eye(TILE, dtype=np.float32) * np.float32(ALPHA / (1.0 - ALPHA))
         ).astype(bf16))

    i0_cols = max(pc[0].shape[1] for pc in per_core)
    i1_cols = max(pc[1].shape[1] for pc in per_core)
    nc = _build(layout, i0_cols, i1_cols, **bkw)

    in_maps = []
    for c in range(M):
        idx0, idx1, dstl, perm_rows = per_core[c]
        h_pad = np.zeros((NPAD, D), bf16)
        h_pad[:len(perm_rows)] = h_bf[perm_rows]
        ids, rows = layout["nodes_by_core"][c]
        h_self = np.zeros((TPC * TILE, D), bf16)
        h_self[rows] = h_bf[ids]
        h_self = np.ascontiguousarray(
            h_self.reshape(TPC, TILE, D).transpose(1, 0, 2).reshape(
                TILE, TPC * D))
        in_maps.append({
            "h_pad": h_pad,
            "h_self": h_self,
            "iota": iota,
            "selfsel": selfsel,
            "idx0": idx0,
            "idx1": idx1,
            "dstl": dstl.astype(bf16),
        })
    return nc, in_maps, layout


def kernel(h, src, dst, **_):
    global LAST_RESULT
    import os
    # NTFF tracing needs an axon hook that is absent in this environment;
    # make sure a stray BASS_TRACE can't break execution.
    os.environ["BASS_NEVER_TRACE"] = "1"
    nc, in_maps, layout = build_and_inputs(h, src, dst)
    res = run_bass_kernel_spmd(nc, in_maps, core_ids=list(range(M)))
    LAST_RESULT = res
    out = np.empty((N, D), np.float32)
    for c in range(M):
        ids, rows = layout["nodes_by_core"][c]
        out[ids] = res.results[c]["out"][rows].astype(np.float32)
    return out

